# revision 24
# baseline (speedup 1.0000x reference)
"""Bidirectional MAMBA Trainium2 kernel.

Sharding (8 cores): (batch 2) x (direction 2) x (d_inner half 2).
Each core runs the full 4-layer chain of its direction on its batch with
E_loc=512 of the 1024 d_inner channels over the full N=1024 sequence
(backward stream is pre-flipped on the host).  Two pair-AllReduces per
layer combine the u@Wx partials and the y@Wout partials.  The selective
scan runs exactly (no approximation) on the DVE via tensor_tensor_scan
with state channels glued along the free dimension (dA zeroed at
segment starts resets the state), channels on partitions (F-layout).

Self-contained: hardcodes all shapes; only needs trn_rl_repo on sys.path.
"""

import os
import sys

import numpy as np

for _p in ("/opt/trn_rl_repo", "/root/.axon_site/_ro/trn_rl_repo"):
    if os.path.isdir(_p) and _p not in sys.path:
        sys.path.insert(0, _p)

import ml_dtypes  # noqa: E402

import concourse.bacc as bacc  # noqa: E402
import concourse.bass as bass  # noqa: E402
import concourse.mybir as mybir  # noqa: E402
import concourse.tile as tile  # noqa: E402
from concourse import bass_utils, library_config  # noqa: E402

F32 = mybir.dt.float32
F32R = mybir.dt.float32r
BF16 = mybir.dt.bfloat16
AF = mybir.ActivationFunctionType
OP = mybir.AluOpType

B, N, DM = 2, 1024, 512          # batch, seq, d_model
EFULL, S, RK, L, K = 1024, 16, 32, 4, 4  # d_inner, d_state, dt_rank, layers, conv
EL = EFULL // 2                  # d_inner per core (tensor-parallel half)
P = 128
KD = DM // P                     # 4 k-tiles over d_model
ET = EL // P                     # 4 tiles over local d_inner
SG = 2                           # s-values per scan group
NG = S // SG                     # 8 scan groups
GF = SG * N                      # glued free size per scan group (2048)
CQ = 256                         # combine token-chunk

_CACHE = {}


def _declare_inputs(nc):
    t = {}

    def d(name, shape, dt=F32):
        t[name] = nc.dram_tensor(name, list(shape), dt, kind="ExternalInput").ap()

    d("x_in", (N, DM))
    d("ident", (P, P))
    d("ones_col", (P, 1))
    d("mask_fw", (P, 2))                 # [:,0]=is_fwd, [:,1]=is_bwd
    d("ln_g", (L, DM)); d("ln_b", (L, DM))
    d("win", (L, DM, 2 * EL), BF16)      # cols = [u-half | z-half]
    d("convd", (L, K, ET, P, P), BF16)   # diagonalized depthwise conv weights
    d("conv_b", (L, EL))
    d("wx", (L, EL, RK + 2 * S), BF16)
    d("wdt", (L, RK, EL), BF16)
    d("bdt", (L, EL))
    d("amat", (L, EL, S))                # A = -exp(Alog) slice
    d("dvec", (L, EL))
    d("wout", (L, EL, DM), BF16)
    d("cg", (2 * DM,)); d("cb", (2 * DM,))
    d("cw", (2 * DM, DM), BF16)
    d("cbias", (DM,))
    out = nc.dram_tensor("out", [N, DM], F32, kind="ExternalOutput").ap()
    return t, out


def _build_program(sim_compat=False):
    key = ("prog", sim_compat)
    if key in _CACHE:
        return _CACHE[key]
    nc = bacc.Bacc("TRN2", target_bir_lowering=False, debug=False, num_devices=8)
    t, out_ap = _declare_inputs(nc)
    with tile.TileContext(nc) as tc:
        _kernel(tc, t, out_ap, sim_compat)
    nc.compile()
    _CACHE[key] = nc
    return nc


def _kernel(tc, t, out_ap, sim_compat=False):
    nc = tc.nc
    import contextlib
    es = contextlib.ExitStack()

    eh_groups = [[0, 1], [2, 3], [4, 5], [6, 7]]       # d_inner-half pairs
    dir_groups = [[0, 2], [1, 3], [4, 6], [5, 7]]      # fwd/bwd pairs

    pers = es.enter_context(tc.tile_pool(name="pers", bufs=1))
    wp = es.enter_context(tc.tile_pool(name="wp", bufs=1))
    sb = es.enter_context(tc.tile_pool(name="sb", bufs=1))
    tp = es.enter_context(tc.tile_pool(name="tp", bufs=3))   # (P,N) f32 scratch
    scn = es.enter_context(tc.tile_pool(name="scn", bufs=2))
    pm = es.enter_context(tc.tile_pool(name="pm", bufs=2, space="PSUM"))
    pt = es.enter_context(tc.tile_pool(name="pt", bufs=2, space="PSUM"))
    ps = es.enter_context(tc.tile_pool(name="ps", bufs=2, space="PSUM"))
    dram = es.enter_context(tc.tile_pool(name="dram", bufs=1, space="DRAM"))

    def t4(name):
        return tp.tile([P, N], F32, tag="t4", name=name)

    def apply_silu(out, psum_in, bias, uid):
        if not sim_compat:
            if bias is None:
                nc.scalar.activation(out, psum_in, AF.Silu)
            else:
                nc.scalar.activation(out, psum_in, AF.Silu, bias=bias)
            return
        tsg = tp.tile([P, N], F32, tag="bcr", bufs=2, name=f"sg{uid}")
        tli = tp.tile([P, N], F32, tag="bcr", bufs=2, name=f"sl{uid}")
        w = psum_in.shape[-1]
        if bias is None:
            nc.scalar.activation(tsg[:, 0:w], psum_in, AF.Sigmoid)
            nc.scalar.activation(tli[:, 0:w], psum_in, AF.Identity)
        else:
            nc.scalar.activation(tsg[:, 0:w], psum_in, AF.Sigmoid, bias=bias)
            nc.scalar.activation(tli[:, 0:w], psum_in, AF.Identity, bias=bias)
        nc.vector.tensor_mul(out, tli[:, 0:w], tsg[:, 0:w])

    # ---- persistent tiles
    x = [pers.tile([P, N], F32, tag=f"x{i}", name=f"x{i}") for i in range(KD)]
    ident_d = pers.tile([P, P], F32, tag="identd", name="ident_d")
    ident = pers.tile([P, P], F32, tag="ident", name="ident")
    ones_d = pers.tile([P, 1], F32, tag="onesd", name="ones_d")
    ones_col = pers.tile([P, 1], F32, tag="ones", name="ones_col")
    ones_bf = pers.tile([P, 1], BF16, tag="onesb", name="ones_bf")
    maskfw = pers.tile([P, 2], F32, tag="maskfw", name="maskfw")
    nc.sync.dma_start(ident_d[:], t["ident"])
    nc.vector.tensor_copy(ident[:], ident_d[:])
    nc.sync.dma_start(ones_d[:], t["ones_col"])
    nc.vector.tensor_copy(ones_col[:], ones_d[:])
    nc.vector.tensor_copy(ones_bf[:], ones_d[:])
    nc.sync.dma_start(maskfw[:], t["mask_fw"])

    # ---- load x (T-layout) and transpose to F-layout
    for i in range(N // P):
        xt = tp.tile([P, DM], F32, tag="tio", bufs=2, name=f"xt{i}")
        nc.sync.dma_start(xt[:], t["x_in"][i * P:(i + 1) * P, :])
        for j in range(KD):
            pshalf = pt.tile([P, P], F32, tag="tp", name=f"tp{i}_{j}")
            nc.tensor.transpose(pshalf[:], xt[:, j * P:(j + 1) * P], ident[:])
            nc.scalar.copy(x[j][:, i * P:(i + 1) * P], pshalf[:])

    def layernorm(xtiles, nkt, gap, bap, hout, ncol, uid):
        """Normalize over the feature (partition) axis.  xtiles: nkt tiles
        (P, ncol) f32; hout: nkt tiles (P, ncol) bf16."""
        nfeat = nkt * P
        nm = sb.tile([1, N], F32, tag="nm", name=f"nm{uid}")
        rstd = sb.tile([1, N], F32, tag="rstd", name=f"rstd{uid}")
        nh = max(1, ncol // 512)
        for h in range(nh):
            sl = slice(h * 512, min((h + 1) * 512, ncol))
            w = sl.stop - sl.start
            sxq = ps.tile([33, 512], F32, tag="st", name=f"sxq{uid}_{h}")
            sx = sxq[0:1]
            sq = sxq[32:33]
            for i in range(nkt):
                xsq = tp.tile([P, N], BF16, tag="xsqb", bufs=2,
                              name=f"xsq{uid}_{h}_{i}")
                nc.scalar.square(xsq[:, 0:w], xtiles[i][:, sl])
                ones_lhs = (ones_col[:] if xtiles[i].dtype == F32
                            else ones_bf[:])
                nc.tensor.matmul(sx[:, 0:w], ones_lhs, xtiles[i][:, sl],
                                 start=(i == 0), stop=(i == nkt - 1))
                nc.tensor.matmul(sq[:, 0:w], ones_bf[:], xsq[:, 0:w],
                                 start=(i == 0), stop=(i == nkt - 1))
            nmh = nm[0:1, sl]
            rsh = rstd[0:1, sl]
            nc.vector.tensor_scalar_mul(nmh, sx[0:1, 0:w], -1.0 / nfeat)
            nc.vector.tensor_scalar_mul(rsh, sq[0:1, 0:w], 1.0 / nfeat)
            # rstd currently E[x^2]; subtract mean^2 (scratch: reuse sq psum)
            nc.vector.tensor_mul(sq[0:1, 0:w], nmh, nmh)
            nc.vector.tensor_sub(rsh, rsh, sq[0:1, 0:w])
            nc.vector.tensor_scalar_add(rsh, rsh, 1e-5)
        nc.scalar.activation(rstd[0:1, 0:ncol], rstd[0:1, 0:ncol], AF.Ln)
        nc.scalar.activation(rstd[0:1, 0:ncol], rstd[0:1, 0:ncol], AF.Exp,
                             scale=-0.5)
        nmb = tp.tile([P, N], F32, tag="bcr", bufs=2, name=f"nmb{uid}")
        rsb = tp.tile([P, N], F32, tag="bcr", bufs=2, name=f"rsb{uid}")

        nc.gpsimd.partition_broadcast(nmb[:, 0:ncol], nm[0:1, 0:ncol])
        nc.gpsimd.partition_broadcast(rsb[:, 0:ncol], rstd[0:1, 0:ncol])
        for i in range(nkt):
            gcol = sb.tile([P, 1], F32, tag="gcol", name=f"gcol{uid}_{i}")
            bcol = sb.tile([P, 1], F32, tag="bcol", name=f"bcol{uid}_{i}")
            nc.sync.dma_start(gcol[:], gap[i * P:(i + 1) * P].unsqueeze(-1))
            nc.sync.dma_start(bcol[:], bap[i * P:(i + 1) * P].unsqueeze(-1))
            t1 = t4(f"lnt1_{uid}_{i}")
            nc.vector.tensor_add(t1[:, 0:ncol], xtiles[i][:, 0:ncol],
                                 nmb[:, 0:ncol])
            nc.vector.tensor_mul(t1[:, 0:ncol], t1[:, 0:ncol], rsb[:, 0:ncol])
            nc.vector.tensor_scalar(hout[i][:, 0:ncol], t1[:, 0:ncol],
                                    gcol[:], bcol[:], op0=OP.mult, op1=OP.add)

    # ================= layers =================
    for l in range(L):
        # -- per-layer weights
        winw = [wp.tile([P, 2 * EL], BF16, tag=f"win{k}", name=f"win{l}_{k}")
                for k in range(KD)]
        for k in range(KD):
            nc.sync.dma_start(winw[k][:], t["win"][l, k * P:(k + 1) * P, :])
        convw = [[wp.tile([P, P], BF16, tag=f"cv{j}_{m}", name=f"cv{l}_{j}_{m}")
                  for m in range(ET)] for j in range(K)]
        for j in range(K):
            for m in range(ET):
                nc.sync.dma_start(convw[j][m][:], t["convd"][l, j, m])
        wxw = [wp.tile([P, RK + 2 * S], BF16, tag=f"wx{k}", name=f"wx{l}_{k}")
               for k in range(ET)]
        for k in range(ET):
            nc.sync.dma_start(wxw[k][:], t["wx"][l, k * P:(k + 1) * P, :])
        wdtw = wp.tile([RK, EL], BF16, tag="wdt", name=f"wdt{l}")
        nc.sync.dma_start(wdtw[:], t["wdt"][l])
        woutw = [wp.tile([P, DM], BF16, tag=f"wo{k}", name=f"wo{l}_{k}")
                 for k in range(ET)]
        for k in range(ET):
            nc.sync.dma_start(woutw[k][:], t["wout"][l, k * P:(k + 1) * P, :])
        amat = [wp.tile([P, S], F32, tag=f"am{m}", name=f"am{l}_{m}")
                for m in range(ET)]
        for m in range(ET):
            nc.sync.dma_start(amat[m][:], t["amat"][l, m * P:(m + 1) * P, :])
        pcol = [[wp.tile([P, 1], F32, tag=f"pc{w}_{m}", name=f"pc{w}{l}_{m}")
                 for m in range(ET)] for w in range(3)]
        for m in range(ET):
            sl = slice(m * P, (m + 1) * P)
            nc.sync.dma_start(pcol[0][m][:], t["bdt"][l, sl].unsqueeze(-1))
            nc.sync.dma_start(pcol[1][m][:], t["conv_b"][l, sl].unsqueeze(-1))
            nc.sync.dma_start(pcol[2][m][:], t["dvec"][l, sl].unsqueeze(-1))
        bdtc, cbc, dvc = pcol

        # -- layernorm
        hln = [sb.tile([P, N], BF16, tag=f"hln{i}", name=f"hln{l}_{i}")
               for i in range(KD)]
        layernorm(x, KD, t["ln_g"][l], t["ln_b"][l], hln, N, f"l{l}")

        # -- Win u-wave + conv + silu
        usi = [sb.tile([P, N], BF16, tag=f"usi{m}", name=f"usi{l}_{m}")
               for m in range(ET)]
        for m in range(ET):
            pu = pm.tile([P, N], F32, tag="gemm", name=f"pu{l}_{m}")
            for hf in range(2):
                fs = slice(hf * 512, (hf + 1) * 512)
                for k in range(KD):
                    nc.tensor.matmul(pu[:, fs], winw[k][:, m * P:(m + 1) * P],
                                     hln[k][:, fs],
                                     start=(k == 0), stop=(k == KD - 1))
            u_sb = tp.tile([P, K - 1 + N], BF16, tag="usb", bufs=2, name=f"usb{l}_{m}")
            nc.vector.memset(u_sb[:, 0:K - 1], 0.0)
            nc.scalar.copy(u_sb[:, K - 1:], pu[:])
            pc = pm.tile([P, N], F32, tag="gemm", name=f"pcv{l}_{m}")
            for hf in range(2):
                for j in range(K):
                    nc.tensor.matmul(pc[:, hf * 512:(hf + 1) * 512], convw[j][m][:],
                                     u_sb[:, hf * 512 + j:hf * 512 + j + 512],
                                     start=(j == 0), stop=(j == K - 1))
            apply_silu(usi[m][:], pc[:], cbc[m][:], f"u{l}_{m}")

        # -- Wx partial GEMM + pair AllReduce (bf16)
        px = pm.tile([RK + 2 * S, N], F32, tag="gemm", name=f"px{l}")
        for hf in range(2):
            fs = slice(hf * 512, (hf + 1) * 512)
            for k in range(ET):
                nc.tensor.matmul(px[:, fs], wxw[k][:], usi[k][:, fs],
                                 start=(k == 0), stop=(k == ET - 1))
        xdbl_part = dram.tile([RK + 2 * S, N], BF16, tag="xdp", name=f"xdp{l}")
        xdbl_sum = dram.tile([RK + 2 * S, N], BF16, tag="xds", name=f"xds{l}")
        pxs = tp.tile([RK + 2 * S, N], BF16, tag="pxsb", bufs=2, name=f"pxs{l}")
        nc.scalar.copy(pxs[:], px[:])
        nc.sync.dma_start(xdbl_part[:], pxs[:])
        nc.gpsimd.collective_compute(
            "AllReduce", OP.add, replica_groups=eh_groups,
            ins=[xdbl_part[:]], outs=[xdbl_sum[:]])

        # -- Win z-wave + silu (overlaps the collective)
        zsi = [sb.tile([P, N], BF16, tag=f"zsi{m}", name=f"zsi{l}_{m}")
               for m in range(ET)]
        for m in range(ET):
            pz = pm.tile([P, N], F32, tag="gemm", name=f"pz{l}_{m}")
            for hf in range(2):
                fs = slice(hf * 512, (hf + 1) * 512)
                for k in range(KD):
                    nc.tensor.matmul(pz[:, fs],
                                     winw[k][:, (ET + m) * P:(ET + m + 1) * P],
                                     hln[k][:, fs],
                                     start=(k == 0), stop=(k == KD - 1))
            apply_silu(zsi[m][:], pz[:], None, f"z{l}_{m}")

        # -- fetch CC result (dt-rank rows only; B/C broadcast straight from DRAM)
        xdbl_bf = sb.tile([RK, N], BF16, tag="xdblb", name=f"xdblb{l}")
        nc.sync.dma_start(xdbl_bf[:], xdbl_sum[0:RK, :])


        # -- dt = softplus(xdbl[:,:RK] @ Wdt + bdt)
        dtb = [sb.tile([P, N], BF16, tag=f"dtb{m}", name=f"dtb{l}_{m}")
               for m in range(ET)]
        spxs = []
        for m in range(ET):
            pd = pm.tile([P, N], F32, tag="gemm", name=f"pd{l}_{m}")
            for hf in range(2):
                fs = slice(hf * 512, (hf + 1) * 512)
                nc.tensor.matmul(pd[:, fs], wdtw[:, m * P:(m + 1) * P],
                                 xdbl_bf[0:RK, fs], start=True, stop=True)
            spx = tp.tile([P, N], BF16, tag="spx", bufs=4, name=f"spx{l}_{m}")
            nc.scalar.activation(spx[:], pd[:], AF.Exp, bias=bdtc[m][:])
            spxs.append(spx)
        for m in range(ET):
            nc.scalar.activation(dtb[m][:], spxs[m][:], AF.Ln, bias=ones_col[:])

        # -- v = dt * usi
        vb = [sb.tile([P, N], BF16, tag=f"vb{m}", name=f"vb{l}_{m}")
              for m in range(ET)]
        for m in range(ET):
            nc.vector.tensor_mul(vb[m][:], dtb[m][:], usi[m][:])

        # -- scan groups
        yred = [sb.tile([P, N], BF16, tag=f"yr{m}", name=f"yr{l}_{m}")
                for m in range(ET)]
        pend = []

        def _flush_red(item):
            fg, fm, fym = item
            if fg == 0:
                nc.vector.tensor_add(yred[fm][:], fym[:, 0:N], fym[:, N:])
            else:
                yg = scn.tile([P, N], BF16, tag="gt", bufs=4,
                              name=f"yg{l}_{fg}_{fm}")
                nc.vector.tensor_add(yg[:], fym[:, 0:N], fym[:, N:])
                nc.vector.tensor_add(yred[fm][:], yred[fm][:], yg[:])

        for g in range(NG):
            bb = scn.tile([P, GF], BF16, tag="bb", name=f"bb{l}_{g}")
            cc = scn.tile([P, GF], BF16, tag="cc", name=f"cc{l}_{g}")
            for si in range(SG):
                s = g * SG + si
                nc.sync.dma_start(
                    bb[:, si * N:(si + 1) * N],
                    xdbl_sum[RK + s:RK + s + 1, :].partition_broadcast(P))
                nc.sync.dma_start(
                    cc[:, si * N:(si + 1) * N],
                    xdbl_sum[RK + S + s:RK + S + s + 1, :].partition_broadcast(P))
            for m in range(ET):
                da = scn.tile([P, GF], BF16, tag="da", name=f"da{l}_{g}_{m}")
                for si in range(SG):
                    s = g * SG + si
                    nc.scalar.activation(da[:, si * N:(si + 1) * N], dtb[m][:],
                                         AF.Exp, scale=amat[m][:, s:s + 1])
                daz = da.rearrange("p (s n) -> p s n", s=SG)
                nc.vector.memset(daz[:, :, 0:1], 0.0)
                dbu = scn.tile([P, GF], BF16, tag="dbu", bufs=3, name=f"dbu{l}_{g}_{m}")
                vv = vb[m].unsqueeze(1).broadcast_to((P, SG, N))
                bb3 = bb.rearrange("p (s n) -> p s n", s=SG)
                db3 = dbu.rearrange("p (s n) -> p s n", s=SG)
                nc.vector.tensor_mul(db3[:], vv, bb3[:])
                hh = scn.tile([P, GF], BF16, tag="dbu", bufs=3, name=f"hh{l}_{g}_{m}")
                nc.vector.tensor_tensor_scan(hh[:], da[:], dbu[:], 0.0,
                                             op0=OP.mult, op1=OP.add)
                ym = scn.tile([P, GF], BF16, tag="dbu", bufs=3, name=f"ym{l}_{g}_{m}")
                nc.gpsimd.tensor_mul(ym[:], hh[:], cc[:])
                pend.append((g, m, ym))
                if len(pend) > 1:
                    _flush_red(pend.pop(0))

        while pend:
            _flush_red(pend.pop(0))

        # -- gate: gated = (yred + usi*D) * zsi
        gated = [scn.tile([P, N], BF16, tag="gt", bufs=4, name=f"gt{l}_{m}")
                 for m in range(ET)]
        for m in range(ET):
            nc.vector.scalar_tensor_tensor(gated[m][:], usi[m][:], dvc[m][:],
                                           yred[m][:], op0=OP.mult, op1=OP.add)
            nc.vector.tensor_mul(gated[m][:], gated[m][:], zsi[m][:])

        # -- Wout partial + pair AllReduce (bf16) + residual add
        dx_part = dram.tile([DM, N], BF16, tag="dxp", name=f"dxp{l}")
        dx_sum = dram.tile([DM, N], BF16, tag="dxs", name=f"dxs{l}")
        for mo in range(KD):
            po = pm.tile([P, N], F32, tag="gemm", name=f"po{l}_{mo}")
            for hf in range(2):
                fs = slice(hf * 512, (hf + 1) * 512)
                for k in range(ET):
                    nc.tensor.matmul(po[:, fs], woutw[k][:, mo * P:(mo + 1) * P],
                                     gated[k][:, fs],
                                     start=(k == 0), stop=(k == ET - 1))
            pos = tp.tile([P, N], BF16, tag="bpd", bufs=2, name=f"pos{l}_{mo}")
            nc.scalar.copy(pos[:], po[:])
            nc.sync.dma_start(dx_part[mo * P:(mo + 1) * P, :], pos[:])
            nc.gpsimd.collective_compute(
                "AllReduce", OP.add, replica_groups=eh_groups,
                ins=[dx_part[mo * P:(mo + 1) * P, :]],
                outs=[dx_sum[mo * P:(mo + 1) * P, :]])
        for mo in range(KD):
            dxs = tp.tile([P, N], BF16, tag="bpd", bufs=2, name=f"dxs{l}_{mo}")
            nc.sync.dma_start(dxs[:], dx_sum[mo * P:(mo + 1) * P, :])
            nc.vector.tensor_add(x[mo][:], x[mo][:], dxs[:])

    # ================= combine =================
    cat_part = dram.tile([DM, N], BF16, tag="catp", name="cat_part")
    cat_sum = dram.tile([2 * DM, N], BF16, tag="cats", name="cat_sum")
    for i in range(KD):
        # contrib = x*is_fwd + flip(x)*is_bwd -- each core sends only its half;
        # AllGather orders fwd (lower rank) before bwd within each dir pair.
        sf = tp.tile([P, N], BF16, tag="bpd", bufs=2, name=f"sf{i}")
        nc.vector.tensor_scalar_mul(sf[:], x[i][:], maskfw[:, 0:1])
        nc.vector.scalar_tensor_tensor(sf[:], x[i][:, ::-1], maskfw[:, 1:2],
                                       sf[:], op0=OP.mult, op1=OP.add)
        nc.sync.dma_start(cat_part[i * P:(i + 1) * P, :], sf[:])
    nc.gpsimd.collective_compute(
        "AllGather", OP.bypass, replica_groups=dir_groups,
        ins=[cat_part[:]], outs=[cat_sum[:]])

    cww = [wp.tile([P, DM], BF16, tag=f"cwt{k}", name=f"cw{k}")
           for k in range(2 * KD)]
    for k in range(2 * KD):
        nc.sync.dma_start(cww[k][:], t["cw"][k * P:(k + 1) * P, :])
    cbias_c = [wp.tile([P, 1], F32, tag=f"cbs{m}", name=f"cbs{m}")
               for m in range(KD)]
    for m in range(KD):
        nc.sync.dma_start(cbias_c[m][:], t["cbias"][m * P:(m + 1) * P].unsqueeze(-1))

    for q in range(N // CQ):
        qs = slice(q * CQ, (q + 1) * CQ)
        xc = [sb.tile([P, CQ], BF16, tag="xc", bufs=9, name=f"xc{q}_{i}")
              for i in range(2 * KD)]
        for i in range(2 * KD):
            nc.sync.dma_start(xc[i][:], cat_sum[i * P:(i + 1) * P, qs])
        hcq = [sb.tile([P, CQ], BF16, tag="hc", bufs=9, name=f"hc{q}_{i}")
               for i in range(2 * KD)]
        layernorm(xc, 2 * KD, t["cg"], t["cb"], hcq, CQ, f"c{q}")
        ot = tp.tile([P, DM], F32, tag="tio", bufs=2, name=f"ot{q}_a")
        ot2 = tp.tile([P, DM], F32, tag="tio", bufs=2, name=f"ot{q}_b")
        for m in range(KD):
            pg = pm.tile([P, N], F32, tag="gemm", name=f"pg{q}_{m}")
            for k in range(2 * KD):
                nc.tensor.matmul(pg[:, 0:CQ], cww[k][:, m * P:(m + 1) * P],
                                 hcq[k][:], start=(k == 0), stop=(k == 2 * KD - 1))
            ogm = t4(f"og{q}_{m}")
            gfn = AF.Identity if sim_compat else AF.Gelu
            nc.scalar.activation(ogm[:, 0:CQ], pg[:, 0:CQ], gfn,
                                 bias=cbias_c[m][:])
            for hh2 in range(CQ // P):
                pts = pt.tile([P, P], F32, tag="tp", name=f"otp{q}_{m}_{hh2}")
                nc.tensor.transpose(
                    pts[:], ogm[:, hh2 * P:(hh2 + 1) * P], ident[:])
                dst = ot if hh2 == 0 else ot2
                nc.scalar.copy(dst[:, m * P:(m + 1) * P], pts[:])
        nc.sync.dma_start(out_ap[q * CQ:q * CQ + P, :], ot[:])
        nc.sync.dma_start(out_ap[q * CQ + P:(q + 1) * CQ, :], ot2[:])

    es.close()


# ----------------------------------------------------------------- host side
def _bf(a):
    return np.asarray(a, dtype=np.float32).astype(ml_dtypes.bfloat16)


def _core_inputs(inputs, b, dirn, e):
    pre = "fwd" if dirn == 0 else "bwd"
    g = lambda n: np.asarray(inputs[pre + "_" + n], dtype=np.float32)
    x = np.asarray(inputs["x"], dtype=np.float32)[b]          # (N, DM)
    if dirn == 1:
        x = x[::-1]
    es = slice(e * EL, (e + 1) * EL)

    win_full = g("Win")                                        # (L, DM, 2*EFULL)
    win = np.concatenate(
        [win_full[:, :, e * EL:(e + 1) * EL],
         win_full[:, :, EFULL + e * EL:EFULL + (e + 1) * EL]], axis=2)

    cw4 = g("conv_w")[:, es, 0, :]                             # (L, EL, K)
    convd = np.zeros((L, K, ET, P, P), np.float32)
    for j in range(K):
        for m in range(ET):
            for l in range(L):
                np.fill_diagonal(convd[l, j, m], cw4[l, m * P:(m + 1) * P, j])

    return {
        "x_in": np.ascontiguousarray(x),
        "ident": np.eye(P, dtype=np.float32),
        "ones_col": np.ones((P, 1), np.float32),
        "mask_fw": np.tile(np.array(
            [1.0, 0.0] if dirn == 0 else [0.0, 1.0], np.float32), (P, 1)),
        "ln_g": g("ln_g"), "ln_b": g("ln_b"),
        "win": _bf(win),
        "convd": _bf(convd),
        "conv_b": g("conv_b")[:, es],
        "wx": _bf(g("Wx")[:, es, :]),
        "wdt": _bf(g("Wdt")[:, :, es]),
        "bdt": g("bdt")[:, es],
        "amat": -np.exp(g("Alog")[:, es, :]),
        "dvec": g("D")[:, es],
        "wout": _bf(g("Wout")[:, es, :]),
        "cg": np.asarray(inputs["cmb_ln_g"], np.float32),
        "cb": np.asarray(inputs["cmb_ln_b"], np.float32),
        "cw": _bf(np.asarray(inputs["cmb_W"], np.float32)),
        "cbias": np.asarray(inputs["cmb_b"], np.float32),
    }


def make_in_maps(inputs):
    in_maps = []
    for b in range(B):
        for dirn in range(2):
            for e in range(2):
                in_maps.append(_core_inputs(inputs, b, dirn, e))
    return in_maps


def kernel(**inputs):
    nc = _build_program()
    res = bass_utils.run_bass_kernel_spmd(nc, make_in_maps(inputs),
                                          list(range(8)))
    out = np.empty((B, N, DM), np.float32)
    for b in range(B):
        out[b] = res.results[b * 4]["out"]
    return out


if __name__ == "__main__":
    nc = _build_program()
    n_inst = sum(len(bb.instructions) for f in nc.m.functions for bb in f.blocks)
    print("program built ok:", n_inst, "instructions")



# revision 28
# speedup vs baseline: 1.1596x; 1.1596x over previous
"""Bidirectional MAMBA Trainium2 kernel.

Sharding (8 cores): (batch 2) x (direction 2) x (d_inner half 2).
Each core runs the full 4-layer chain of its direction on its batch with
E_loc=512 of the 1024 d_inner channels over the full N=1024 sequence
(backward stream is pre-flipped on the host).  Two pair-AllReduces per
layer combine the u@Wx partials and the y@Wout partials.  The selective
scan runs exactly (no approximation) on the DVE via tensor_tensor_scan
with state channels glued along the free dimension (dA zeroed at
segment starts resets the state), channels on partitions (F-layout).

Self-contained: hardcodes all shapes; only needs trn_rl_repo on sys.path.
"""

import os
import sys

import numpy as np

for _p in ("/opt/trn_rl_repo", "/root/.axon_site/_ro/trn_rl_repo"):
    if os.path.isdir(_p) and _p not in sys.path:
        sys.path.insert(0, _p)

import ml_dtypes  # noqa: E402

import concourse.bacc as bacc  # noqa: E402
import concourse.bass as bass  # noqa: E402
import concourse.mybir as mybir  # noqa: E402
import concourse.tile as tile  # noqa: E402
from concourse import bass_utils, library_config  # noqa: E402

F32 = mybir.dt.float32
F32R = mybir.dt.float32r
BF16 = mybir.dt.bfloat16
AF = mybir.ActivationFunctionType
OP = mybir.AluOpType

B, N, DM = 2, 1024, 512          # batch, seq, d_model
EFULL, S, RK, L, K = 1024, 16, 32, 4, 4  # d_inner, d_state, dt_rank, layers, conv
EL = EFULL // 2                  # d_inner per core (tensor-parallel half)
P = 128
KD = DM // P                     # 4 k-tiles over d_model
ET = EL // P                     # 4 tiles over local d_inner
SG = 2                           # s-values per scan group
NG = S // SG                     # 8 scan groups
GF = SG * N                      # glued free size per scan group (2048)
CQ = 256                         # combine token-chunk

_CACHE = {}


def _declare_inputs(nc):
    t = {}

    def d(name, shape, dt=F32):
        t[name] = nc.dram_tensor(name, list(shape), dt, kind="ExternalInput").ap()

    d("x_in", (N, DM))
    d("ident", (P, P))
    d("ones_col", (P, 1))
    d("mask_fw", (P, 2))                 # [:,0]=is_fwd, [:,1]=is_bwd
    d("ln_g", (L, DM)); d("ln_b", (L, DM))
    d("win", (L, DM, 2 * EL), BF16)      # cols = [u-half | z-half]
    d("convd", (L, K, ET, P, P), BF16)   # diagonalized depthwise conv weights
    d("conv_b", (L, EL))
    d("wx", (L, EL, RK + 2 * S), BF16)
    d("wdt", (L, RK, EL), BF16)
    d("bdt", (L, EL))
    d("amat", (L, EL, S))                # A = -exp(Alog) slice
    d("dvec", (L, EL))
    d("wout", (L, EL, DM), BF16)
    d("cg", (2 * DM,)); d("cb", (2 * DM,))
    d("cw", (2 * DM, DM), BF16)
    d("cbias", (DM,))
    out = nc.dram_tensor("out", [N, DM], F32, kind="ExternalOutput").ap()
    return t, out


def _build_program(sim_compat=False):
    key = ("prog", sim_compat)
    if key in _CACHE:
        return _CACHE[key]
    nc = bacc.Bacc("TRN2", target_bir_lowering=False, debug=False, num_devices=8)
    t, out_ap = _declare_inputs(nc)
    with tile.TileContext(nc) as tc:
        _kernel(tc, t, out_ap, sim_compat)
    nc.compile()
    _CACHE[key] = nc
    return nc


def _kernel(tc, t, out_ap, sim_compat=False):
    nc = tc.nc
    import contextlib
    es = contextlib.ExitStack()

    eh_groups = [[0, 1], [2, 3], [4, 5], [6, 7]]       # d_inner-half pairs
    dir_groups = [[0, 2], [1, 3], [4, 6], [5, 7]]      # fwd/bwd pairs

    pers = es.enter_context(tc.tile_pool(name="pers", bufs=1))
    wp = es.enter_context(tc.tile_pool(name="wp", bufs=1))
    sb = es.enter_context(tc.tile_pool(name="sb", bufs=1))
    tp = es.enter_context(tc.tile_pool(name="tp", bufs=3))   # (P,N) f32 scratch
    scn = es.enter_context(tc.tile_pool(name="scn", bufs=2))
    pm = es.enter_context(tc.tile_pool(name="pm", bufs=2, space="PSUM"))
    pt = es.enter_context(tc.tile_pool(name="pt", bufs=2, space="PSUM"))
    ps = es.enter_context(tc.tile_pool(name="ps", bufs=2, space="PSUM"))
    dram = es.enter_context(tc.tile_pool(name="dram", bufs=1, space="DRAM"))

    def t4(name):
        return tp.tile([P, N], F32, tag="t4", name=name)

    def apply_silu(out, psum_in, bias, uid):
        if not sim_compat:
            if bias is None:
                nc.scalar.activation(out, psum_in, AF.Silu)
            else:
                nc.scalar.activation(out, psum_in, AF.Silu, bias=bias)
            return
        tsg = tp.tile([P, N], F32, tag="bcr", bufs=2, name=f"sg{uid}")
        tli = tp.tile([P, N], F32, tag="bcr", bufs=2, name=f"sl{uid}")
        w = psum_in.shape[-1]
        if bias is None:
            nc.scalar.activation(tsg[:, 0:w], psum_in, AF.Sigmoid)
            nc.scalar.activation(tli[:, 0:w], psum_in, AF.Identity)
        else:
            nc.scalar.activation(tsg[:, 0:w], psum_in, AF.Sigmoid, bias=bias)
            nc.scalar.activation(tli[:, 0:w], psum_in, AF.Identity, bias=bias)
        nc.vector.tensor_mul(out, tli[:, 0:w], tsg[:, 0:w])

    # ---- persistent tiles
    x = [pers.tile([P, N], F32, tag=f"x{i}", name=f"x{i}") for i in range(KD)]
    ident_d = pers.tile([P, P], F32, tag="identd", name="ident_d")
    ident = pers.tile([P, P], F32, tag="ident", name="ident")
    ones_d = pers.tile([P, 1], F32, tag="onesd", name="ones_d")
    ones_col = pers.tile([P, 1], F32, tag="ones", name="ones_col")
    ones_bf = pers.tile([P, 1], BF16, tag="onesb", name="ones_bf")
    maskfw = pers.tile([P, 2], F32, tag="maskfw", name="maskfw")
    nc.sync.dma_start(ident_d[:], t["ident"])
    nc.vector.tensor_copy(ident[:], ident_d[:])
    nc.sync.dma_start(ones_d[:], t["ones_col"])
    nc.vector.tensor_copy(ones_col[:], ones_d[:])
    nc.vector.tensor_copy(ones_bf[:], ones_d[:])
    nc.sync.dma_start(maskfw[:], t["mask_fw"])

    # ---- load x (T-layout) and transpose to F-layout
    for i in range(N // P):
        xt = tp.tile([P, DM], F32, tag="tio", bufs=2, name=f"xt{i}")
        nc.sync.dma_start(xt[:], t["x_in"][i * P:(i + 1) * P, :])
        for j in range(KD):
            pshalf = pt.tile([P, P], F32, tag="tp", name=f"tp{i}_{j}")
            nc.tensor.transpose(pshalf[:], xt[:, j * P:(j + 1) * P], ident[:])
            nc.scalar.copy(x[j][:, i * P:(i + 1) * P], pshalf[:])

    def layernorm(xtiles, nkt, gap, bap, hout, ncol, uid):
        """Normalize over the feature (partition) axis.  xtiles: nkt tiles
        (P, ncol) f32; hout: nkt tiles (P, ncol) bf16."""
        nfeat = nkt * P
        nm = sb.tile([1, N], BF16, tag="nm", name=f"nm{uid}")
        rstd = sb.tile([1, N], BF16, tag="rstd", name=f"rstd{uid}")
        nh = max(1, ncol // 512)
        for h in range(nh):
            sl = slice(h * 512, min((h + 1) * 512, ncol))
            w = sl.stop - sl.start
            sxq = ps.tile([33, 512], F32, tag="st", name=f"sxq{uid}_{h}")
            sx = sxq[0:1]
            sq = sxq[32:33]
            for i in range(nkt):
                xsq = tp.tile([P, N], BF16, tag="xsqb", bufs=2,
                              name=f"xsq{uid}_{h}_{i}")
                nc.scalar.square(xsq[:, 0:w], xtiles[i][:, sl])
                ones_lhs = (ones_col[:] if xtiles[i].dtype == F32
                            else ones_bf[:])
                nc.tensor.matmul(sx[:, 0:w], ones_lhs, xtiles[i][:, sl],
                                 start=(i == 0), stop=(i == nkt - 1))
                nc.tensor.matmul(sq[:, 0:w], ones_bf[:], xsq[:, 0:w],
                                 start=(i == 0), stop=(i == nkt - 1))
            nmh = nm[0:1, sl]
            rsh = rstd[0:1, sl]
            nc.vector.tensor_scalar_mul(nmh, sx[0:1, 0:w], -1.0 / nfeat)
            nc.vector.tensor_scalar_mul(rsh, sq[0:1, 0:w], 1.0 / nfeat)
            # rstd currently E[x^2]; subtract mean^2 (scratch: reuse sq psum)
            nc.vector.tensor_mul(sq[0:1, 0:w], nmh, nmh)
            nc.vector.tensor_sub(rsh, rsh, sq[0:1, 0:w])
            nc.vector.tensor_scalar_add(rsh, rsh, 1e-5)
        nc.scalar.activation(rstd[0:1, 0:ncol], rstd[0:1, 0:ncol], AF.Ln)
        nc.scalar.activation(rstd[0:1, 0:ncol], rstd[0:1, 0:ncol], AF.Exp,
                             scale=-0.5)
        nmb = tp.tile([P, N], BF16, tag="bcrb", bufs=2, name=f"nmb{uid}")
        rsb = tp.tile([P, N], BF16, tag="bcrb", bufs=2, name=f"rsb{uid}")

        nc.gpsimd.partition_broadcast(nmb[:, 0:ncol], nm[0:1, 0:ncol])
        nc.gpsimd.partition_broadcast(rsb[:, 0:ncol], rstd[0:1, 0:ncol])
        for i in range(nkt):
            gcol = sb.tile([P, 1], F32, tag="gcol", name=f"gcol{uid}_{i}")
            bcol = sb.tile([P, 1], F32, tag="bcol", name=f"bcol{uid}_{i}")
            nc.sync.dma_start(gcol[:], gap[i * P:(i + 1) * P].unsqueeze(-1))
            nc.sync.dma_start(bcol[:], bap[i * P:(i + 1) * P].unsqueeze(-1))
            t1 = t4(f"lnt1_{uid}_{i}")
            nc.vector.tensor_add(t1[:, 0:ncol], xtiles[i][:, 0:ncol],
                                 nmb[:, 0:ncol])
            nc.vector.tensor_mul(t1[:, 0:ncol], t1[:, 0:ncol], rsb[:, 0:ncol])
            nc.vector.tensor_scalar(hout[i][:, 0:ncol], t1[:, 0:ncol],
                                    gcol[:], bcol[:], op0=OP.mult, op1=OP.add)

    # ================= layers =================
    for l in range(L):
        # -- per-layer weights
        winw = [wp.tile([P, 2 * EL], BF16, tag=f"win{k}", name=f"win{l}_{k}")
                for k in range(KD)]
        for k in range(KD):
            nc.sync.dma_start(winw[k][:], t["win"][l, k * P:(k + 1) * P, :])
        convw = [[wp.tile([P, P], BF16, tag=f"cv{j}_{m}", name=f"cv{l}_{j}_{m}")
                  for m in range(ET)] for j in range(K)]
        for j in range(K):
            for m in range(ET):
                nc.sync.dma_start(convw[j][m][:], t["convd"][l, j, m])
        wxw = [wp.tile([P, RK + 2 * S], BF16, tag=f"wx{k}", name=f"wx{l}_{k}")
               for k in range(ET)]
        for k in range(ET):
            nc.sync.dma_start(wxw[k][:], t["wx"][l, k * P:(k + 1) * P, :])
        wdtw = wp.tile([RK, EL], BF16, tag="wdt", name=f"wdt{l}")
        nc.sync.dma_start(wdtw[:], t["wdt"][l])
        woutw = [wp.tile([P, DM], BF16, tag=f"wo{k}", name=f"wo{l}_{k}")
                 for k in range(ET)]
        for k in range(ET):
            nc.sync.dma_start(woutw[k][:], t["wout"][l, k * P:(k + 1) * P, :])
        amat = [wp.tile([P, S], F32, tag=f"am{m}", name=f"am{l}_{m}")
                for m in range(ET)]
        for m in range(ET):
            nc.sync.dma_start(amat[m][:], t["amat"][l, m * P:(m + 1) * P, :])
        pcol = [[wp.tile([P, 1], F32, tag=f"pc{w}_{m}", name=f"pc{w}{l}_{m}")
                 for m in range(ET)] for w in range(3)]
        for m in range(ET):
            sl = slice(m * P, (m + 1) * P)
            nc.sync.dma_start(pcol[0][m][:], t["bdt"][l, sl].unsqueeze(-1))
            nc.sync.dma_start(pcol[1][m][:], t["conv_b"][l, sl].unsqueeze(-1))
            nc.sync.dma_start(pcol[2][m][:], t["dvec"][l, sl].unsqueeze(-1))
        bdtc, cbc, dvc = pcol

        # -- layernorm
        hln = [sb.tile([P, N], BF16, tag=f"hln{i}", name=f"hln{l}_{i}")
               for i in range(KD)]
        layernorm(x, KD, t["ln_g"][l], t["ln_b"][l], hln, N, f"l{l}")

        # -- Win u-wave + conv + silu
        usi = [sb.tile([P, N], BF16, tag=f"usi{m}", name=f"usi{l}_{m}")
               for m in range(ET)]
        for m in range(ET):
            pu = pm.tile([P, N], F32, tag="gemm", name=f"pu{l}_{m}")
            for hf in range(2):
                fs = slice(hf * 512, (hf + 1) * 512)
                for k in range(KD):
                    nc.tensor.matmul(pu[:, fs], winw[k][:, m * P:(m + 1) * P],
                                     hln[k][:, fs],
                                     start=(k == 0), stop=(k == KD - 1))
            u_sb = tp.tile([P, K - 1 + N], BF16, tag="usb", bufs=2, name=f"usb{l}_{m}")
            nc.vector.memset(u_sb[:, 0:K - 1], 0.0)
            nc.scalar.copy(u_sb[:, K - 1:], pu[:])
            pc = pm.tile([P, N], F32, tag="gemm", name=f"pcv{l}_{m}")
            for hf in range(2):
                for j in range(K):
                    nc.tensor.matmul(pc[:, hf * 512:(hf + 1) * 512], convw[j][m][:],
                                     u_sb[:, hf * 512 + j:hf * 512 + j + 512],
                                     start=(j == 0), stop=(j == K - 1))
            apply_silu(usi[m][:], pc[:], cbc[m][:], f"u{l}_{m}")

        # -- Wx partial GEMM + pair AllReduce (bf16)
        px = pm.tile([RK + 2 * S, N], F32, tag="gemm", name=f"px{l}")
        for hf in range(2):
            fs = slice(hf * 512, (hf + 1) * 512)
            for k in range(ET):
                nc.tensor.matmul(px[:, fs], wxw[k][:], usi[k][:, fs],
                                 start=(k == 0), stop=(k == ET - 1))
        xdbl_part = dram.tile([RK + 2 * S, N], BF16, tag="xdp", name=f"xdp{l}")
        xdbl_sum = dram.tile([RK + 2 * S, N], BF16, tag="xds", name=f"xds{l}")
        pxs = tp.tile([RK + 2 * S, N], BF16, tag="pxsb", bufs=2, name=f"pxs{l}")
        nc.scalar.copy(pxs[:], px[:])
        nc.sync.dma_start(xdbl_part[:], pxs[:])
        nc.gpsimd.collective_compute(
            "AllReduce", OP.add, replica_groups=eh_groups,
            ins=[xdbl_part[:]], outs=[xdbl_sum[:]])

        # -- Win z-wave + silu (overlaps the collective)
        zsi = [sb.tile([P, N], BF16, tag=f"zsi{m}", name=f"zsi{l}_{m}")
               for m in range(ET)]
        for m in range(ET):
            pz = pm.tile([P, N], F32, tag="gemm", name=f"pz{l}_{m}")
            for hf in range(2):
                fs = slice(hf * 512, (hf + 1) * 512)
                for k in range(KD):
                    nc.tensor.matmul(pz[:, fs],
                                     winw[k][:, (ET + m) * P:(ET + m + 1) * P],
                                     hln[k][:, fs],
                                     start=(k == 0), stop=(k == KD - 1))
            apply_silu(zsi[m][:], pz[:], None, f"z{l}_{m}")

        # -- fetch CC result (dt-rank rows only; B/C broadcast straight from DRAM)
        xdbl_bf = sb.tile([RK, N], BF16, tag="xdblb", name=f"xdblb{l}")
        nc.sync.dma_start(xdbl_bf[:], xdbl_sum[0:RK, :])


        # -- dt = softplus(xdbl[:,:RK] @ Wdt + bdt)
        dtb = [sb.tile([P, N], BF16, tag=f"dtb{m}", name=f"dtb{l}_{m}")
               for m in range(ET)]
        spxs = []
        for m in range(ET):
            pd = pm.tile([P, N], F32, tag="gemm", name=f"pd{l}_{m}")
            for hf in range(2):
                fs = slice(hf * 512, (hf + 1) * 512)
                nc.tensor.matmul(pd[:, fs], wdtw[:, m * P:(m + 1) * P],
                                 xdbl_bf[0:RK, fs], start=True, stop=True)
            spx = tp.tile([P, N], BF16, tag="spx", bufs=4, name=f"spx{l}_{m}")
            nc.scalar.activation(spx[:], pd[:], AF.Exp, bias=bdtc[m][:])
            spxs.append(spx)
        for m in range(ET):
            nc.scalar.activation(dtb[m][:], spxs[m][:], AF.Ln, bias=ones_col[:])

        # -- v = dt * usi
        vb = [sb.tile([P, N], BF16, tag=f"vb{m}", name=f"vb{l}_{m}")
              for m in range(ET)]
        for m in range(ET):
            nc.vector.tensor_mul(vb[m][:], dtb[m][:], usi[m][:])

        # -- scan groups
        yred = [sb.tile([P, N], BF16, tag=f"yr{m}", name=f"yr{l}_{m}")
                for m in range(ET)]
        pend = []

        def _flush_red(item):
            fg, fm, fym = item
            if fg == 0:
                nc.vector.tensor_add(yred[fm][:], fym[:, 0:N], fym[:, N:])
            else:
                yg = scn.tile([P, N], BF16, tag="gt", bufs=4,
                              name=f"yg{l}_{fg}_{fm}")
                nc.vector.tensor_add(yg[:], fym[:, 0:N], fym[:, N:])
                nc.vector.tensor_add(yred[fm][:], yred[fm][:], yg[:])

        for g in range(NG):
            bb = scn.tile([P, GF], BF16, tag="bb", name=f"bb{l}_{g}")
            cc = scn.tile([P, GF], BF16, tag="cc", name=f"cc{l}_{g}")
            for si in range(SG):
                s = g * SG + si
                nc.sync.dma_start(
                    bb[:, si * N:(si + 1) * N],
                    xdbl_sum[RK + s:RK + s + 1, :].partition_broadcast(P))
                nc.sync.dma_start(
                    cc[:, si * N:(si + 1) * N],
                    xdbl_sum[RK + S + s:RK + S + s + 1, :].partition_broadcast(P))
            for m in range(ET):
                da = scn.tile([P, GF], BF16, tag="da", name=f"da{l}_{g}_{m}")
                for si in range(SG):
                    s = g * SG + si
                    nc.scalar.activation(da[:, si * N:(si + 1) * N], dtb[m][:],
                                         AF.Exp, scale=amat[m][:, s:s + 1])
                daz = da.rearrange("p (s n) -> p s n", s=SG)
                nc.vector.memset(daz[:, :, 0:1], 0.0)
                dbu = scn.tile([P, GF], BF16, tag="dbu", bufs=3, name=f"dbu{l}_{g}_{m}")
                vv = vb[m].unsqueeze(1).broadcast_to((P, SG, N))
                bb3 = bb.rearrange("p (s n) -> p s n", s=SG)
                db3 = dbu.rearrange("p (s n) -> p s n", s=SG)
                nc.vector.tensor_mul(db3[:], vv, bb3[:])
                hh = scn.tile([P, GF], BF16, tag="dbu", bufs=3, name=f"hh{l}_{g}_{m}")
                nc.vector.tensor_tensor_scan(hh[:], da[:], dbu[:], 0.0,
                                             op0=OP.mult, op1=OP.add)
                ym = scn.tile([P, GF], BF16, tag="ymt", bufs=2, name=f"ym{l}_{g}_{m}")
                nc.gpsimd.tensor_mul(ym[:], hh[:], cc[:])
                pend.append((g, m, ym))
                if len(pend) > 1:
                    _flush_red(pend.pop(0))

        while pend:
            _flush_red(pend.pop(0))

        # -- gate: gated = (yred + usi*D) * zsi
        gated = [scn.tile([P, N], BF16, tag="gt", bufs=4, name=f"gt{l}_{m}")
                 for m in range(ET)]
        for m in range(ET):
            nc.vector.scalar_tensor_tensor(gated[m][:], usi[m][:], dvc[m][:],
                                           yred[m][:], op0=OP.mult, op1=OP.add)
            nc.vector.tensor_mul(gated[m][:], gated[m][:], zsi[m][:])

        # -- Wout partial + pair AllReduce (bf16) + residual add
        dx_part = dram.tile([DM, N], BF16, tag="dxp", name=f"dxp{l}")
        dx_sum = dram.tile([DM, N], BF16, tag="dxs", name=f"dxs{l}")
        for mo in range(KD):
            po = pm.tile([P, N], F32, tag="gemm", name=f"po{l}_{mo}")
            for hf in range(2):
                fs = slice(hf * 512, (hf + 1) * 512)
                for k in range(ET):
                    nc.tensor.matmul(po[:, fs], woutw[k][:, mo * P:(mo + 1) * P],
                                     gated[k][:, fs],
                                     start=(k == 0), stop=(k == ET - 1))
            pos = tp.tile([P, N], BF16, tag="bpd", bufs=2, name=f"pos{l}_{mo}")
            nc.scalar.copy(pos[:], po[:])
            nc.sync.dma_start(dx_part[mo * P:(mo + 1) * P, :], pos[:])
            nc.gpsimd.collective_compute(
                "AllReduce", OP.add, replica_groups=eh_groups,
                ins=[dx_part[mo * P:(mo + 1) * P, :]],
                outs=[dx_sum[mo * P:(mo + 1) * P, :]])
        for mo in range(KD):
            dxs = tp.tile([P, N], BF16, tag="bpd", bufs=2, name=f"dxs{l}_{mo}")
            nc.sync.dma_start(dxs[:], dx_sum[mo * P:(mo + 1) * P, :])
            nc.vector.tensor_add(x[mo][:], x[mo][:], dxs[:])

    # ================= combine =================
    cat_part = dram.tile([DM, N], BF16, tag="catp", name="cat_part")
    cat_sum = dram.tile([2 * DM, N], BF16, tag="cats", name="cat_sum")
    for i in range(KD):
        # contrib = x*is_fwd + flip(x)*is_bwd -- each core sends only its half;
        # AllGather orders fwd (lower rank) before bwd within each dir pair.
        sf = tp.tile([P, N], BF16, tag="bpd", bufs=2, name=f"sf{i}")
        nc.vector.tensor_scalar_mul(sf[:], x[i][:], maskfw[:, 0:1])
        nc.vector.scalar_tensor_tensor(sf[:], x[i][:, ::-1], maskfw[:, 1:2],
                                       sf[:], op0=OP.mult, op1=OP.add)
        nc.sync.dma_start(cat_part[i * P:(i + 1) * P, :], sf[:])
    nc.gpsimd.collective_compute(
        "AllGather", OP.bypass, replica_groups=dir_groups,
        ins=[cat_part[:]], outs=[cat_sum[:]])

    cww = [wp.tile([P, DM], BF16, tag=f"cwt{k}", name=f"cw{k}")
           for k in range(2 * KD)]
    for k in range(2 * KD):
        nc.sync.dma_start(cww[k][:], t["cw"][k * P:(k + 1) * P, :])
    cbias_c = [wp.tile([P, 1], F32, tag=f"cbs{m}", name=f"cbs{m}")
               for m in range(KD)]
    for m in range(KD):
        nc.sync.dma_start(cbias_c[m][:], t["cbias"][m * P:(m + 1) * P].unsqueeze(-1))

    for q in range(N // CQ):
        qs = slice(q * CQ, (q + 1) * CQ)
        xc = [sb.tile([P, CQ], BF16, tag="xc", bufs=9, name=f"xc{q}_{i}")
              for i in range(2 * KD)]
        for i in range(2 * KD):
            nc.sync.dma_start(xc[i][:], cat_sum[i * P:(i + 1) * P, qs])
        hcq = [sb.tile([P, CQ], BF16, tag="hc", bufs=9, name=f"hc{q}_{i}")
               for i in range(2 * KD)]
        layernorm(xc, 2 * KD, t["cg"], t["cb"], hcq, CQ, f"c{q}")
        ot = tp.tile([P, DM], F32, tag="tio", bufs=2, name=f"ot{q}_a")
        ot2 = tp.tile([P, DM], F32, tag="tio", bufs=2, name=f"ot{q}_b")
        for m in range(KD):
            pg = pm.tile([P, N], F32, tag="gemm", name=f"pg{q}_{m}")
            for k in range(2 * KD):
                nc.tensor.matmul(pg[:, 0:CQ], cww[k][:, m * P:(m + 1) * P],
                                 hcq[k][:], start=(k == 0), stop=(k == 2 * KD - 1))
            ogm = t4(f"og{q}_{m}")
            gfn = AF.Identity if sim_compat else AF.Gelu
            nc.scalar.activation(ogm[:, 0:CQ], pg[:, 0:CQ], gfn,
                                 bias=cbias_c[m][:])
            for hh2 in range(CQ // P):
                pts = pt.tile([P, P], F32, tag="tp", name=f"otp{q}_{m}_{hh2}")
                nc.tensor.transpose(
                    pts[:], ogm[:, hh2 * P:(hh2 + 1) * P], ident[:])
                dst = ot if hh2 == 0 else ot2
                nc.scalar.copy(dst[:, m * P:(m + 1) * P], pts[:])
        nc.sync.dma_start(out_ap[q * CQ:q * CQ + P, :], ot[:])
        nc.sync.dma_start(out_ap[q * CQ + P:(q + 1) * CQ, :], ot2[:])

    es.close()


# ----------------------------------------------------------------- host side
def _bf(a):
    return np.asarray(a, dtype=np.float32).astype(ml_dtypes.bfloat16)


def _core_inputs(inputs, b, dirn, e):
    pre = "fwd" if dirn == 0 else "bwd"
    g = lambda n: np.asarray(inputs[pre + "_" + n], dtype=np.float32)
    x = np.asarray(inputs["x"], dtype=np.float32)[b]          # (N, DM)
    if dirn == 1:
        x = x[::-1]
    es = slice(e * EL, (e + 1) * EL)

    win_full = g("Win")                                        # (L, DM, 2*EFULL)
    win = np.concatenate(
        [win_full[:, :, e * EL:(e + 1) * EL],
         win_full[:, :, EFULL + e * EL:EFULL + (e + 1) * EL]], axis=2)

    cw4 = g("conv_w")[:, es, 0, :]                             # (L, EL, K)
    convd = np.zeros((L, K, ET, P, P), np.float32)
    for j in range(K):
        for m in range(ET):
            for l in range(L):
                np.fill_diagonal(convd[l, j, m], cw4[l, m * P:(m + 1) * P, j])

    return {
        "x_in": np.ascontiguousarray(x),
        "ident": np.eye(P, dtype=np.float32),
        "ones_col": np.ones((P, 1), np.float32),
        "mask_fw": np.tile(np.array(
            [1.0, 0.0] if dirn == 0 else [0.0, 1.0], np.float32), (P, 1)),
        "ln_g": g("ln_g"), "ln_b": g("ln_b"),
        "win": _bf(win),
        "convd": _bf(convd),
        "conv_b": g("conv_b")[:, es],
        "wx": _bf(g("Wx")[:, es, :]),
        "wdt": _bf(g("Wdt")[:, :, es]),
        "bdt": g("bdt")[:, es],
        "amat": -np.exp(g("Alog")[:, es, :]),
        "dvec": g("D")[:, es],
        "wout": _bf(g("Wout")[:, es, :]),
        "cg": np.asarray(inputs["cmb_ln_g"], np.float32),
        "cb": np.asarray(inputs["cmb_ln_b"], np.float32),
        "cw": _bf(np.asarray(inputs["cmb_W"], np.float32)),
        "cbias": np.asarray(inputs["cmb_b"], np.float32),
    }


def make_in_maps(inputs):
    in_maps = []
    for b in range(B):
        for dirn in range(2):
            for e in range(2):
                in_maps.append(_core_inputs(inputs, b, dirn, e))
    return in_maps


def kernel(**inputs):
    nc = _build_program()
    res = bass_utils.run_bass_kernel_spmd(nc, make_in_maps(inputs),
                                          list(range(8)))
    out = np.empty((B, N, DM), np.float32)
    for b in range(B):
        out[b] = res.results[b * 4]["out"]
    return out


if __name__ == "__main__":
    nc = _build_program()
    n_inst = sum(len(bb.instructions) for f in nc.m.functions for bb in f.blocks)
    print("program built ok:", n_inst, "instructions")

